# revision 21
# baseline (speedup 1.0000x reference)
"""Bahdanau attention scorer for Trainium2, 8-core data-parallel over batch.

scores[b, s] = v_a . tanh(W_s @ enc_outs[s, b] + W_t @ dec_out[b] + b_t)

Shapes (fixed): enc_outs (2048, 64, 512) f32, dec_out (64, 512) f32,
W_s/W_t (512, 512) f32, b_t/v_a (512,) f32 -> scores (64, 2048) f32.

Sharding: batch 64 -> 8 cores x 8 batches. Small params replicated.

Host prep does all layout work so the device kernel is pure streaming:
  * enc is pre-transposed per core to [block][hc][128 h][512 tokens] bf16
    so matmul contraction (over h) needs no on-device transposes.
  * dec bias (W_t @ dec + b_t) is computed host-side in f64 -> f32.
Per-core device pipeline, one (b, s-block) tile of 512 tokens per step:
  1. 4 chunk DMAs HBM -> SBUF (bf16 [128 h, 512 tok] each); the first
     block's loads are interleaved with the W_s chunk loads so the first
     matmul issues ~5 us sooner.
  2. PE: 4x4 matmuls psum[ac] (128 a, 512 tok) += W_sT[hc,ac].T @ x[hc].
  3. ACT: tanh(psum + bias[b, ac]) -> bf16.
  4. DVE: scale each chunk by v_a, summing incrementally as chunks land.
  5. PE ones-matmul reduces 128 partitions -> psumV (1, 512) -> DVE copy
     to SBUF -> DMA out.
The partition-reduce matmul for block k is emitted after block k+2's main
matmuls so the PE never stalls waiting for the ACT/DVE chain (measured:
PE busy 83%+ of exec with <4 us of internal gaps; the kernel is within
~10% of the 8-core bf16 PE roofline of ~109 us/core for the main GEMM).
"""

import sys

sys.path.insert(0, "/opt/trn_rl_repo")

import numpy as np
import ml_dtypes

import concourse.bass as bass
import concourse.mybir as mybir
import concourse.tile as tile
from concourse import bacc
from concourse.bass_utils import run_bass_kernel_spmd

S, B, H, A = 2048, 64, 512, 512
NCORES = 8
BL = B // NCORES          # local batches per core
HC = H // 128             # h chunks
AC = A // 128             # a chunks
SBLK = 512                # tokens per block
NSB = S // SBLK           # s blocks per batch row
NBLK = BL * NSB           # blocks per core

F32 = mybir.dt.float32
BF16 = mybir.dt.bfloat16
BF16_NP = ml_dtypes.bfloat16

_CACHE = {}


def build_kernel():
    nc = bacc.Bacc("TRN2", target_bir_lowering=False, debug=False,
                   num_devices=NCORES)

    enc_d = nc.dram_tensor("enc", [NBLK * HC * 128, SBLK], BF16,
                           kind="ExternalInput")
    wst_d = nc.dram_tensor("wst", [128, HC * A], BF16, kind="ExternalInput")
    bias_d = nc.dram_tensor("bias", [128, AC * BL], F32, kind="ExternalInput")
    va_d = nc.dram_tensor("va", [128, AC], F32, kind="ExternalInput")
    out_d = nc.dram_tensor("scores", [1, BL * S], F32, kind="ExternalOutput")

    with tile.TileContext(nc) as tc:
        with tc.tile_pool(name="consts", bufs=1) as constp:
            wst_sb = constp.tile([128, HC * A], BF16, tag="wst")
            va_sb = constp.tile([128, AC], F32, tag="va")
            bias_sb = constp.tile([128, AC * BL], F32, tag="bias")
            ones_sb = constp.tile([128, 1], BF16, tag="ones")
            nc.gpsimd.memset(ones_sb[:], 1.0)
            warm_sb = constp.tile([128, 256], BF16, tag="warm")
            nc.gpsimd.memset(warm_sb[:], 1.0)

            with (
                tc.tile_pool(name="xin", bufs=3 * HC) as xinp,
                tc.tile_pool(name="act", bufs=5 * AC) as actp,
                tc.tile_pool(name="stage", bufs=6) as stagep,
                tc.tile_pool(name="ps_mm", bufs=6, space="PSUM") as mmp,
                tc.tile_pool(name="ps_v", bufs=2, space="PSUM") as pvp,
            ):
                pending = []  # (vm_tile, b, sb, c0, w) awaiting reduce

                # warm-up: the PE p-state ramps only while the engine is
                # busy, and the first ~10 us are DMA fill where it idles.
                # Dummy matmuls during the fill pre-ramp the clock so the
                # first real matmuls run near full speed.
                for _ in range(7):
                    wps = pvp.tile([1, SBLK], F32, tag="pv")
                    nc.tensor.matmul(wps[:, 0:256], ones_sb[:], warm_sb[:],
                                     start=True, stop=True)

                def emit_reduce(vm, b, sb, c0, w):
                    psV = pvp.tile([1, SBLK], F32, tag="pv")
                    nc.tensor.matmul(psV[:, 0:w], ones_sb[:], vm[:, 0:w],
                                     start=True, stop=True)
                    stg = stagep.tile([1, SBLK], F32, tag="stage")
                    nc.vector.tensor_copy(stg[:, 0:w], psV[:, 0:w])
                    nc.sync.dma_start(
                        out_d[0:1, b * S + sb * SBLK + c0:
                              b * S + sb * SBLK + c0 + w], stg[:, 0:w])

                def emit_postmm(psM, b, sb, c0, w):
                    """tanh + v_a scale + incremental chunk-sum for columns
                    [c0, c0+w). Summing as chunks arrive keeps the post-mul
                    tail to a single DVE add."""
                    vacc = None
                    for ac in range(AC):
                        th = actp.tile([128, SBLK], BF16, tag="tanh")
                        nc.scalar.activation(
                            th[:, 0:w], psM[ac][:, c0:c0 + w],
                            mybir.ActivationFunctionType.Tanh,
                            bias=bias_sb[:, ac * BL + b: ac * BL + b + 1],
                        )
                        vm = actp.tile([128, SBLK], BF16, tag="vm")
                        nc.vector.tensor_scalar_mul(
                            vm[:, 0:w], th[:, 0:w], va_sb[:, ac:ac + 1])
                        if vacc is None:
                            vacc = vm
                        else:
                            nc.vector.tensor_add(vacc[:, 0:w], vacc[:, 0:w],
                                                 vm[:, 0:w])
                    pending.append((vacc, b, sb, c0, w))

                for blk in range(NBLK):
                    b, sb = divmod(blk, NSB)
                    xc = []
                    for hc in range(HC):
                        if blk == 0:
                            # interleave weight-chunk loads with the first
                            # block's loads so the first matmul starts asap
                            nc.sync.dma_start(
                                wst_sb[:, hc * A:(hc + 1) * A],
                                wst_d[:, hc * A:(hc + 1) * A])
                        t = xinp.tile([128, SBLK], BF16, tag=f"x{hc}")
                        r0 = (blk * HC + hc) * 128
                        nc.sync.dma_start(t[:], enc_d[r0:r0 + 128, :])
                        xc.append(t)
                    if blk == 0:
                        nc.sync.dma_start(va_sb[:], va_d[:])
                        nc.sync.dma_start(bias_sb[:], bias_d[:])

                    psM = []
                    for ac in range(AC):
                        ps = mmp.tile([128, SBLK], F32, tag="mm")
                        psM.append(ps)
                        for hc in range(HC):
                            nc.tensor.matmul(
                                ps[:],
                                wst_sb[:, hc * A + ac * 128:
                                       hc * A + ac * 128 + 128],
                                xc[hc][:],
                                start=(hc == 0), stop=(hc == HC - 1),
                            )

                    # reduce for an earlier block now that two more blocks'
                    # matmuls are queued ahead of it on the PE — by then its
                    # ACT/DVE chain has certainly drained, so no PE stall
                    while len(pending) >= 2:
                        emit_reduce(*pending.pop(0))

                    emit_postmm(psM, b, sb, 0, SBLK)

                for p in pending:
                    emit_reduce(*p)

    nc.compile()
    return nc


def _prep_host(dec_out, enc_outs, W_s, W_t, b_t, v_a):
    # W_s.T laid out as [128 h-part, HC * A]
    wst = np.ascontiguousarray(
        W_s.T.reshape(HC, 128, A).transpose(1, 0, 2).reshape(128, HC * A)
    ).astype(BF16_NP)
    # dec bias, exact on host: bias[a, b] = (W_t @ dec[b] + b_t)[a]
    bias = (dec_out.astype(np.float64) @ W_t.T.astype(np.float64)
            + b_t.astype(np.float64)).T.astype(np.float32)   # (A, B)
    va4 = np.ascontiguousarray(
        v_a.reshape(AC, 128).T).astype(np.float32)           # (128, AC)

    enc_bf = enc_outs.astype(BF16_NP)                        # (S, B, H)
    in_maps = []
    for k in range(NCORES):
        # -> [b, sb, hc, p, c] -> rows ((b*NSB+sb)*HC+hc)*128 + p
        e = enc_bf[:, k * BL:(k + 1) * BL, :]
        e6 = e.reshape(NSB, SBLK, BL, HC, 128).transpose(2, 0, 3, 4, 1)
        enc_l = np.ascontiguousarray(e6).reshape(NBLK * HC * 128, SBLK)
        bl = bias[:, k * BL:(k + 1) * BL]                    # (A, BL)
        bias_l = np.ascontiguousarray(
            bl.reshape(AC, 128, BL).transpose(1, 0, 2).reshape(128, AC * BL))
        in_maps.append({
            "enc": enc_l,
            "wst": wst,
            "bias": bias_l,
            "va": va4,
        })
    return in_maps


def kernel(dec_out, enc_outs, W_s, W_t, b_t, v_a, trace=False):
    dec_out = np.asarray(dec_out)
    enc_outs = np.asarray(enc_outs)
    if "nc" not in _CACHE:
        _CACHE["nc"] = build_kernel()
    nc = _CACHE["nc"]
    in_maps = _prep_host(dec_out, enc_outs,
                         np.asarray(W_s), np.asarray(W_t),
                         np.asarray(b_t), np.asarray(v_a))
    res = run_bass_kernel_spmd(nc, in_maps, core_ids=list(range(NCORES)),
                               trace=trace)
    out = np.concatenate(
        [res.results[k]["scores"].reshape(BL, S) for k in range(NCORES)],
        axis=0).astype(np.float32)
    if trace:
        _CACHE["last_result"] = res
    return out
